# revision 5
# baseline (speedup 1.0000x reference)
"""Trainium2 Bass kernel for nn_BilinearDecoder (embedding gather + bilinear scores).

For each edge e (head h, tail t, negatives n_s[:, 0:16] = 8 tail-negs + 8 head-negs):
    hW[e]  = embed[h[e]] @ W
    tWT[e] = embed[t[e]] @ W.T
    pos[e]     = hW[e] . embed[t[e]]
    neg1[e, k] = hW[e] . embed[n_t[e, k]]
    neg2[e, k] = tWT[e] . embed[n_h[e, k]]

Data-parallel over edges across 8 NeuronCores (embed table + W replicated).

Device strategy (per core), built around the int16-indexed `dma_gather`
fast path (0.34 ns/descriptor SWDGE):
  Phase 1: host bins the h (resp. t) indices by 32768-row table chunk;
    chunked dma_gathers pull the rows in binned order; PE transposes +
    matmuls produce hW (resp. tWT) rows, written to DRAM scratch in the
    same binned order ("position" order).
  Phase 2: every (edge, dot-slot) pair is a "ref" (17 per edge: pos uses
    the t row, 8 tail-negs, 8 head-negs). Refs are binned by
    (operand-table, data chunk). Per 4096-ref piece: one dma_gather for
    the data rows from embed (int16 chunk-local ids) and one dma_gather
    for the operand rows from the phase-1 scratch (int16 position ids —
    positions < 32K by construction), then a fused DVE multiply +
    grouped reduce produces the scores in position order. The host
    un-permutes scores back to (edge, column).

The Bass program is constructed after the inputs are seen, so all bin
sizes are static (max over the 8 cores, padded to multiples of 128).
"""
import contextlib

import numpy as np

import concourse.bacc as bacc
import concourse.tile as tile
from concourse import bass, mybir, bass_utils
from concourse.masks import make_identity

E = 100000
N = 200000
D = 128
K = 16
NCORES = 8
P = 128
CH = 32768                 # int16-addressable rows per gather chunk
NCHUNK = -(-N // CH)       # 7
PIECE = 4096               # refs per phase-2 piece

EDGES_PER_CORE = E // NCORES              # 12500
TILES_PER_CORE = -(-EDGES_PER_CORE // P)  # 98
EC = TILES_PER_CORE * P                   # 12544 padded edges/core

LAST_EXEC_NS = None


def _set_problem(e, n, ncores):
    """Reconfigure module for a smaller test problem."""
    global E, N, NCORES, NCHUNK, EDGES_PER_CORE, TILES_PER_CORE, EC
    E, N, NCORES = e, n, ncores
    NCHUNK = -(-N // CH)
    EDGES_PER_CORE = E // NCORES
    TILES_PER_CORE = -(-EDGES_PER_CORE // P)
    EC = TILES_PER_CORE * P


def _rup(x, m):
    return -(-x // m) * m


def _wrap16(flat_i16):
    """[R] int16 -> [128, R/16] wrapped in 16 partitions, replicated x8."""
    r = flat_i16.shape[0]
    w = flat_i16.reshape(r // 16, 16).T
    return np.ascontiguousarray(np.tile(w, (8, 1)))


def _bin_positions(vals, caps, bases):
    """vals [n] ints; returns (pos [n], local [n]) with pos = bases[chunk] + rank
    (rank = stable order within chunk), local = vals - chunk*CH."""
    chunk = vals // CH
    pos = np.empty(vals.shape[0], np.int64)
    for c in range(len(caps)):
        sel = np.flatnonzero(chunk == c)
        assert len(sel) <= caps[c]
        pos[sel] = bases[c] + np.arange(len(sel))
    return pos, (vals - chunk * CH).astype(np.int16)


def _build_program(caps1, caps2, seg2_meta, r1, r2):
    """caps1: 7 phase-1 segment capacities (per table); caps2: 11 phase-2
    segment capacities; seg2_meta: list of (operand_id 0=h/1=t, chunk)."""
    nc = bacc.Bacc("TRN2", target_bir_lowering=False)
    embed = nc.dram_tensor("embed", [N, D], mybir.dt.float32, kind="ExternalInput")
    w_in = nc.dram_tensor("w", [D, D], mybir.dt.float32, kind="ExternalInput")
    idx1h = nc.dram_tensor("idx1h", [P, r1 // 16], mybir.dt.int16, kind="ExternalInput")
    idx1t = nc.dram_tensor("idx1t", [P, r1 // 16], mybir.dt.int16, kind="ExternalInput")
    idx2d = nc.dram_tensor("idx2d", [P, r2 // 16], mybir.dt.int16, kind="ExternalInput")
    idx2o = nc.dram_tensor("idx2o", [P, r2 // 16], mybir.dt.int16, kind="ExternalInput")
    out = nc.dram_tensor("scores", [P, r2 // P], mybir.dt.float32,
                         kind="ExternalOutput")

    bases1 = np.concatenate([[0], np.cumsum(caps1)])[:-1]
    bases2 = np.concatenate([[0], np.cumsum(caps2)])[:-1]

    with tile.TileContext(nc) as tc:
        with contextlib.ExitStack() as ctx:
            sb = ctx.enter_context(tc.tile_pool(name="sb", bufs=1))
            dram = ctx.enter_context(tc.tile_pool(name="dram", bufs=1, space="DRAM"))
            ps = ctx.enter_context(tc.tile_pool(name="ps", bufs=2, space="PSUM"))
            sbm = ctx.enter_context(tc.tile_pool(name="sbm", bufs=2))

            w_s = sb.tile([D, D], mybir.dt.float32)
            wt_s = sb.tile([D, D], mybir.dt.float32)
            ident = sb.tile([P, P], mybir.dt.float32)
            i1h_s = sb.tile([P, r1 // 16], mybir.dt.int16)
            i1t_s = sb.tile([P, r1 // 16], mybir.dt.int16)
            i2d_s = sb.tile([P, r2 // 16], mybir.dt.int16)
            i2o_s = sb.tile([P, r2 // 16], mybir.dt.int16)
            scores_s = sb.tile([P, r2 // P], mybir.dt.float32)

            nc.sync.dma_start(out=w_s[:], in_=w_in[:])
            nc.sync.dma_start(out=i1h_s[:], in_=idx1h[:])
            nc.sync.dma_start(out=i1t_s[:], in_=idx1t[:])
            nc.sync.dma_start(out=i2d_s[:], in_=idx2d[:])
            nc.sync.dma_start(out=i2o_s[:], in_=idx2o[:])
            make_identity(nc, ident[:])
            wt_p = ps.tile([D, D], mybir.dt.float32, space="PSUM", tag="tr_p")
            nc.tensor.transpose(out=wt_p[:], in_=w_s[:], identity=ident[:])
            nc.vector.tensor_copy(out=wt_s[:], in_=wt_p[:])

            hw_scr = dram.tile([r1, D], mybir.dt.float32, name="hw_scr")
            twt_scr = dram.tile([r1, D], mybir.dt.float32, name="twt_scr")

            # ---------------- Phase 1 ----------------
            for tbl, (i1_s, rhs, scr) in enumerate(
                    [(i1h_s, w_s, hw_scr), (i1t_s, wt_s, twt_scr)]):
                with tc.tile_pool(name=f"g1p{tbl}", bufs=1) as g1pool:
                    g1 = g1pool.tile([P, r1 // P, D], mybir.dt.float32,
                                     name=f"g1_{tbl}")
                    for c in range(NCHUNK):
                        cap = int(caps1[c])
                        if cap == 0:
                            continue
                        b = int(bases1[c])
                        nrow = min(CH, N - CH * c)
                        nc.gpsimd.dma_gather(
                            out_ap=g1[:, b // P:(b + cap) // P, :],
                            in_ap=embed[CH * c: CH * c + nrow, :],
                            idxs_ap=i1_s[:, b // 16:(b + cap) // 16],
                            num_idxs=cap, num_idxs_reg=cap, elem_size=D,
                            single_packet=cap <= 1008)
                    for i in range(r1 // P):
                        tr_p = ps.tile([P, P], mybir.dt.float32, space="PSUM",
                                       tag="tr_p", name=f"tr_p_{tbl}_{i}")
                        nc.tensor.transpose(out=tr_p[:], in_=g1[:, i, :],
                                            identity=ident[:])
                        tr_s = sbm.tile([P, P], mybir.dt.float32, tag="tr_s",
                                        name=f"tr_s_{tbl}_{i}")
                        nc.scalar.copy(out=tr_s[:], in_=tr_p[:])
                        mm_p = ps.tile([P, D], mybir.dt.float32, space="PSUM",
                                       tag="mm_p", name=f"mm_p_{tbl}_{i}")
                        nc.tensor.matmul(out=mm_p[:], lhsT=tr_s[:], rhs=rhs[:],
                                         start=True, stop=True)
                        mm_s = sbm.tile([P, D], mybir.dt.float32, tag="mm_s",
                                        name=f"mm_s_{tbl}_{i}")
                        nc.vector.tensor_copy(out=mm_s[:], in_=mm_p[:])
                        nc.sync.dma_start(out=scr[P * i:P * (i + 1), :],
                                          in_=mm_s[:])

            # ---------------- Phase 2 ----------------
            with contextlib.ExitStack() as ctx2:
                d2pool = ctx2.enter_context(tc.tile_pool(name="d2", bufs=2))
                o2pool = ctx2.enter_context(tc.tile_pool(name="o2", bufs=2))
                m2pool = ctx2.enter_context(tc.tile_pool(name="m2", bufs=2))
                pc = 0
                for si, (op_id, c) in enumerate(seg2_meta):
                    cap = int(caps2[si])
                    b = int(bases2[si])
                    nrow = min(CH, N - CH * c)
                    scr = hw_scr if op_id == 0 else twt_scr
                    for q0 in range(b, b + cap, PIECE):
                        n = min(PIECE, b + cap - q0)
                        nb = n // P
                        d2 = d2pool.tile([P, PIECE // P, D], mybir.dt.float32,
                                         tag="d2", name=f"d2_{pc}")
                        o2 = o2pool.tile([P, PIECE // P, D], mybir.dt.float32,
                                         tag="o2", name=f"o2_{pc}")
                        m2 = m2pool.tile([P, PIECE // P, D], mybir.dt.float32,
                                         tag="m2", name=f"m2_{pc}")
                        nc.gpsimd.dma_gather(
                            out_ap=d2[:, :nb, :],
                            in_ap=embed[CH * c: CH * c + nrow, :],
                            idxs_ap=i2d_s[:, q0 // 16:(q0 + n) // 16],
                            num_idxs=n, num_idxs_reg=n, elem_size=D,
                            single_packet=n <= 1008)
                        nc.gpsimd.dma_gather(
                            out_ap=o2[:, :nb, :],
                            in_ap=scr[:],
                            idxs_ap=i2o_s[:, q0 // 16:(q0 + n) // 16],
                            num_idxs=n, num_idxs_reg=n, elem_size=D,
                            single_packet=n <= 1008)
                        nc.vector.tensor_tensor(
                            out=m2[:, :nb, :], in0=d2[:, :nb, :],
                            in1=o2[:, :nb, :], op=mybir.AluOpType.mult)
                        nc.vector.tensor_reduce(
                            out=scores_s[:, q0 // P:q0 // P + nb],
                            in_=m2[:, :nb, :], axis=mybir.AxisListType.X,
                            op=mybir.AluOpType.add)
                        pc += 1

            nc.sync.dma_start(out=out[:], in_=scores_s[:])
    nc.compile()
    return nc


def kernel(embed, score_matrix, h, t, n_s):
    global LAST_EXEC_NS
    embed = np.ascontiguousarray(np.asarray(embed, dtype=np.float32))
    w = np.ascontiguousarray(np.asarray(score_matrix, dtype=np.float32))
    h = np.asarray(h).astype(np.int64).reshape(E)
    t = np.asarray(t).astype(np.int64).reshape(E)
    n_s = np.asarray(n_s).astype(np.int64).reshape(E, K)

    # ---- per-core sharding + padding ----
    cores = []
    for c in range(NCORES):
        sl = slice(c * EDGES_PER_CORE, (c + 1) * EDGES_PER_CORE)
        hc = np.zeros(EC, np.int64)
        tc_ = np.zeros(EC, np.int64)
        nsc = np.zeros((EC, K), np.int64)
        hc[:EDGES_PER_CORE] = h[sl]
        tc_[:EDGES_PER_CORE] = t[sl]
        nsc[:EDGES_PER_CORE] = n_s[sl]
        cores.append((hc, tc_, nsc))

    # ---- phase-1 capacities (max over cores) ----
    cnt1 = np.zeros((NCORES, 2, NCHUNK), np.int64)
    for ci, (hc, tc_, _) in enumerate(cores):
        cnt1[ci, 0] = np.bincount(hc // CH, minlength=NCHUNK)
        cnt1[ci, 1] = np.bincount(tc_ // CH, minlength=NCHUNK)
    caps1 = np.array([_rup(int(cnt1[:, :, c].max()), P) for c in range(NCHUNK)])
    r1 = int(caps1.sum())
    assert r1 < 32768
    bases1 = np.concatenate([[0], np.cumsum(caps1)])[:-1]

    # ---- phase-2 refs: data index per (edge, col); col 0 = t-row (pos),
    # cols 1-8 = n_t, cols 9-16 = n_h. Operand: hW for cols 0-8, tWT else.
    seg2_meta = [(0, c) for c in range(NCHUNK)] + [(1, c) for c in range(min(4, NCHUNK))]
    nseg2 = len(seg2_meta)
    seg_of = {m: i for i, m in enumerate(seg2_meta)}

    cnt2 = np.zeros((NCORES, nseg2), np.int64)
    core_data = []
    for ci, (hc, tc_, nsc) in enumerate(cores):
        dat = np.concatenate([tc_[:, None], nsc], axis=1)  # [EC, 17]
        opid = np.zeros((EC, 17), np.int64)
        opid[:, 9:] = 1
        segk = opid * NCHUNK + dat // CH  # seg order: (0,c) x NCHUNK, (1,c) x 4
        core_data.append((dat, opid, segk))
        cnt2[ci] = np.bincount(segk.reshape(-1), minlength=nseg2)
    caps2 = np.array([_rup(int(cnt2[:, s].max()), P) for s in range(nseg2)])
    r2 = int(caps2.sum())
    bases2 = np.concatenate([[0], np.cumsum(caps2)])[:-1]

    # ---- build per-core host arrays ----
    in_maps = []
    qmaps = []
    for ci, (hc, tc_, nsc) in enumerate(cores):
        i1h = np.zeros(r1, np.int16)
        i1t = np.zeros(r1, np.int16)
        pos_h, loc_h = _bin_positions(hc, caps1, bases1)
        pos_t, loc_t = _bin_positions(tc_, caps1, bases1)
        i1h[pos_h] = loc_h
        i1t[pos_t] = loc_t

        dat, opid, segk = core_data[ci]
        flat_dat = dat.reshape(-1)
        flat_seg = segk.reshape(-1)
        flat_e = np.repeat(np.arange(EC), 17)
        i2d = np.zeros(r2, np.int16)
        i2o = np.zeros(r2, np.int16)
        q = np.empty(EC * 17, np.int64)
        for s in range(nseg2):
            sel = np.flatnonzero(flat_seg == s)
            assert len(sel) <= caps2[s]
            qs = bases2[s] + np.arange(len(sel))
            q[sel] = qs
            _, c = seg2_meta[s]
            i2d[qs] = (flat_dat[sel] - c * CH).astype(np.int16)
            pos_tbl = pos_h if seg2_meta[s][0] == 0 else pos_t
            i2o[qs] = pos_tbl[flat_e[sel]].astype(np.int16)
        qmaps.append(q.reshape(EC, 17))

        in_maps.append({
            "embed": embed, "w": w,
            "idx1h": _wrap16(i1h), "idx1t": _wrap16(i1t),
            "idx2d": _wrap16(i2d), "idx2o": _wrap16(i2o),
        })

    nc = _build_program(caps1, caps2, seg2_meta, r1, r2)
    import time
    t0 = time.perf_counter()
    res = bass_utils.run_bass_kernel_spmd(nc, in_maps, core_ids=list(range(NCORES)))
    LAST_EXEC_NS = int((time.perf_counter() - t0) * 1e9)

    # ---- unpermute ----
    outs = []
    for ci in range(NCORES):
        sc = res.results[ci]["scores"]             # [128, r2/128]
        lin = sc.T.reshape(-1)                     # position q = col*128+p
        full = lin[qmaps[ci]]                      # [EC, 17]
        outs.append(full[:EDGES_PER_CORE])
    allsc = np.concatenate(outs, axis=0).astype(np.float32)
    pos_score = np.ascontiguousarray(allsc[:, 0:1])
    neg_score = np.ascontiguousarray(allsc[:, 1:17])
    return pos_score, neg_score
